# revision 16
# baseline (speedup 1.0000x reference)
"""Trainium2 Bass kernel for ViT-style attention with relative position bias.

Module (per batch b, head h):
    qkv = x @ qkv_w.T + cat(q_bias, 0, v_bias)
    attn = softmax(scale * q @ k.T + bias[h])          bias = rel_pos_table[rel_pos_index]
    out  = (attn @ v) @ proj_w.T + proj_b

Distribution: pure data-parallel over batch — 8 NeuronCores x 8 batches each,
no collectives. Each core runs an identical SPMD program on its batch shard.

Device-side layout strategy (per core, all intermediates SBUF-resident bf16):
  - Host pre-transposes x and the weights so no on-device transposes are needed.
  - qk^T matmul produces Q^T/K^T feature-major [64, tokens] slices directly.
  - V matmul produces V token-major [tokens, 64] (the AV contraction needs
    keys on the partition axis), with a ones column appended per head so the
    AV matmul also emits the softmax denominator row for free.
  - Scores are computed transposed, S^T[j, i] = K[j] . Q[i]; softmax uses
    exp(s) * exp(bias) (no max-subtraction: logits are O(3), fp32/bf16 safe);
    exp(bias) is an input-derived constant computed host-side.
  - AV^T [64+1, 197] = Vplus^T @ expS^T is feature-major, feeding the proj
    matmul without transposes; the denominator lands in partition 64.
  - Normalization: reciprocal of the denominator row, DMA-broadcast across 64
    partitions, fused into the PSUM->SBUF evacuation multiply.
Tokens are padded 1576->1664 (13*128) so all matmul m-tiles are full 128 rows.
"""

import numpy as np
import ml_dtypes

import concourse.bass as bass
import concourse.bacc as bacc
import concourse.mybir as mybir
import concourse.tile as tile
from concourse import bass_utils

F32 = mybir.dt.float32
BF16 = mybir.dt.bfloat16

N_CORES = 8
B = 64
B_LOC = B // N_CORES          # 8 batches per core
N = 197                       # tokens per batch
C = 768
H = 12
HD = 64
SCALE = HD ** -0.5
NTOK = B_LOC * N              # 1576
NPAD = 1664                   # 13 * 128
KT = 6                        # 768 / 128 contraction tiles
QK_NT = 4                     # n-tiles over padded tokens
QK_TW = NPAD // QK_NT         # 416

_CACHE = {}


def _build(has_v_bias, has_p_bias, has_q_bias):
    nc = bacc.Bacc("TRN2", target_bir_lowering=False, debug=False)

    xT_d = nc.dram_tensor("xT", [C, NPAD], BF16, kind="ExternalInput")
    wqk_d = nc.dram_tensor("wqk", [C, 2 * C], BF16, kind="ExternalInput")
    wv_d = nc.dram_tensor("wv", [C, C], BF16, kind="ExternalInput")
    wp_d = nc.dram_tensor("wp", [C, C], BF16, kind="ExternalInput")
    expb_d = nc.dram_tensor("expb", [H // 2, 128, 4 * N], BF16, kind="ExternalInput")
    if has_q_bias:
        qb_d = nc.dram_tensor("qb", [128, KT], F32, kind="ExternalInput")
    if has_v_bias:
        vb_d = nc.dram_tensor("vb", [1, C], BF16, kind="ExternalInput")
    if has_p_bias:
        pb_d = nc.dram_tensor("pb", [1, C], BF16, kind="ExternalInput")
    out_d = nc.dram_tensor("out", [NTOK, C], F32, kind="ExternalOutput")

    with tile.TileContext(nc) as tc:
        with (
            tc.tile_pool(name="singles", bufs=1) as singles,
            tc.tile_pool(name="expwork", bufs=3) as expwork,
            tc.tile_pool(name="normwork", bufs=2) as normwork,
            tc.tile_pool(name="outstage", bufs=3) as outstage,
        ):
            # ---- persistent SBUF tensors ----
            wp_sb = singles.tile([128, KT, C], BF16)
            expb_sb = singles.tile([128, H // 2, 4 * N], BF16)
            qkT_sb = singles.tile([128, 12, NPAD], BF16)      # m-tiles 0..5 = Q^T, 6..11 = K^T
            # per-head blocks [V_h 64 | ones 64]; the ones half makes each AV
            # matmul replicate the softmax denominator into PSUM partitions
            # 64:128 (lhsT = [V_h | ones], 128 contiguous columns)
            V_sb = singles.tile([128, 2 * B_LOC, H, 2 * HD], BF16)
            aoT_sb = singles.tile([128, KT, NPAD], BF16)      # attn-out^T, proj stationary

            for kt in range(KT):
                nc.sync.dma_start(out=wp_sb[:, kt, :], in_=wp_d.ap()[kt * 128:(kt + 1) * 128, :])
            for hp in range(H // 2):
                nc.sync.dma_start(out=expb_sb[:, hp, :], in_=expb_d.ap()[hp])
            if has_q_bias:
                qb_sb = singles.tile([128, KT], F32)
                nc.sync.dma_start(out=qb_sb, in_=qb_d.ap())
            if has_v_bias or has_p_bias:
                ones_row = singles.tile([1, NPAD], BF16)
                nc.vector.memset(ones_row, 1.0)
            if has_p_bias:
                pb_sb = singles.tile([1, C], BF16)
                nc.sync.dma_start(out=pb_sb, in_=pb_d.ap())

            # ones blocks for the AV denominator trick
            for s in range(2 * B_LOC):
                nc.vector.memset(V_sb[:, s, :, HD:2 * HD], 1.0)
            # proj reads padded token columns of attn-out^T; keep them finite
            nc.vector.memset(aoT_sb[:, :, NTOK:NPAD], 0.0)

            # ---- merged qk^T + V + attention, interleaved per head pair ----
            with tc.tile_pool(name="wqkpool", bufs=1) as wqkpool:
                wqk_sb = wqkpool.tile([128, KT, 2 * C], BF16)
                xT_sb = wqkpool.tile([128, KT, NPAD], BF16)
                for kt in range(KT):
                    nc.sync.dma_start(out=xT_sb[:, kt, :], in_=xT_d.ap()[kt * 128:(kt + 1) * 128, :])
                    nc.sync.dma_start(out=wqk_sb[:, kt, :], in_=wqk_d.ap()[kt * 128:(kt + 1) * 128, :])

                with (
                    tc.tile_pool(name="ps_qk", bufs=2, space="PSUM") as ps_qk,
                    tc.tile_pool(name="ps_s", bufs=1, space="PSUM") as ps_s_pool,
                    tc.tile_pool(name="ps_av", bufs=3, space="PSUM") as ps_av_pool,
                ):
                    def qk_mtile(mt):
                        for nt in range(QK_NT):
                            pq = ps_qk.tile([128, QK_TW], F32, name="psqk", tag="psqk")
                            for kt in range(KT):
                                nc.tensor.matmul(
                                    pq,
                                    lhsT=wqk_sb[:, kt, mt * 128:(mt + 1) * 128],
                                    rhs=xT_sb[:, kt, nt * QK_TW:(nt + 1) * QK_TW],
                                    start=(kt == 0), stop=(kt == KT - 1),
                                )
                            dst = qkT_sb[:, mt, nt * QK_TW:(nt + 1) * QK_TW]
                            if has_q_bias and mt < KT:
                                nc.any.tensor_scalar_add(dst, pq, qb_sb[:, mt:mt + 1])
                            else:
                                nc.any.tensor_copy(out=dst, in_=pq)

                    def v_block(b, wv_sb, vb_sb):
                        # token-major V for one batch, psum borrowed from ps_qk
                        for jc in range(2):
                            m = 128 if jc == 0 else N - 128
                            tok0 = b * N + jc * 128
                            for nt in range(2):
                                pv = ps_qk.tile([128, QK_TW], F32, name="psqk", tag="psqk")
                                for kt in range(KT):
                                    nc.tensor.matmul(
                                        pv[0:m, 0:C // 2],
                                        lhsT=xT_sb[:, kt, tok0:tok0 + m],
                                        rhs=wv_sb[:, kt, nt * (C // 2):(nt + 1) * (C // 2)],
                                        start=(kt == 0),
                                        stop=(kt == KT - 1 and not has_v_bias),
                                    )
                                if has_v_bias:
                                    nc.tensor.matmul(
                                        pv[0:m, 0:C // 2],
                                        lhsT=ones_row[:, tok0:tok0 + m],
                                        rhs=vb_sb[:, nt * (C // 2):(nt + 1) * (C // 2)],
                                        start=False, stop=True,
                                    )
                                nc.any.tensor_copy(
                                    out=V_sb[0:m, b * 2 + jc, nt * 6:(nt + 1) * 6, 0:HD],
                                    in_=pv[0:m, 0:C // 2].rearrange("p (g f) -> p g f", g=KT),
                                )

                    def attn_front(hp, b, mul_eng):
                        av = ps_av_pool.tile([128, 2 * N], F32, name="av", tag="av")
                        q0 = b * N
                        # Both heads' scores in one 2-bank PSUM tile (head hh at
                        # columns hh*512 + [0, 394)) so one bank-hopping AP
                        # covers the pair in a single EXP. Even/odd heads live
                        # at base partitions 0/64, so their k=64 score matmuls
                        # occupy disjoint PE row groups and overlap.
                        ps_s2 = ps_s_pool.tile([128, 1024], F32, name="ps_s2", tag="ps_s2")
                        for jc in range(2):
                            for hh in range(2):
                                h = hp * 2 + hh
                                pbase = (h % 2) * 64
                                nc.tensor.matmul(
                                    ps_s2[:, hh * 512 + jc * N:hh * 512 + (jc + 1) * N],
                                    lhsT=qkT_sb[pbase:pbase + 64, 6 + h // 2,
                                                q0 + jc * 128:q0 + jc * 128 + 128],
                                    rhs=qkT_sb[pbase:pbase + 64, h // 2, q0:q0 + N],
                                    start=(jc == 0), stop=(jc == 1),
                                )
                        expS2 = expwork.tile([128, 4 * N], BF16, tag="expS2")
                        nc.scalar.activation(
                            expS2.rearrange("p (g w) -> p g w", w=2 * N),
                            ps_s2.rearrange("p (g w) -> p g w", w=512)[:, :, 0:2 * N],
                            mybir.ActivationFunctionType.Exp, scale=SCALE,
                        )
                        expST2 = expwork.tile([128, 4 * N], BF16, tag="expST2")
                        mul_eng.tensor_mul(expST2, expS2, expb_sb[:, hp, :])
                        for hh in range(2):
                            h = hp * 2 + hh
                            for jc in range(2):
                                jm = 128 if jc == 0 else N - 128
                                nc.tensor.matmul(
                                    av[:, hh * N:(hh + 1) * N],
                                    lhsT=V_sb[0:jm, b * 2 + jc, h, :],
                                    rhs=expST2[0:jm, hh * 2 * N + jc * N:hh * 2 * N + (jc + 1) * N],
                                    start=(hh == 0 and jc == 0),
                                    stop=(hh == 1 and jc == 1),
                                )
                        return av

                    def attn_back(hp, b, av):
                        denom = normwork.tile([64, 2 * N], F32, tag="denom")
                        nc.scalar.copy(out=denom, in_=av[64:128, :])
                        recipB = normwork.tile([64, 2 * N], F32, tag="recipB")
                        nc.vector.reciprocal_approx_fast(recipB, denom)
                        for hh in range(2):
                            h = hp * 2 + hh
                            dst = aoT_sb[(h % 2) * 64:(h % 2) * 64 + 64, h // 2, b * N:b * N + N]
                            nc.vector.tensor_mul(
                                dst, av[0:HD, hh * N:(hh + 1) * N], recipB[:, hh * N:(hh + 1) * N]
                            )

                    DEPTH = 2
                    inflight = []
                    pair_i = 0

                    def do_pair(hp, b, mul_eng):
                        inflight.append((hp, b, attn_front(hp, b, mul_eng)))
                        if len(inflight) > DEPTH:
                            attn_back(*inflight.pop(0))

                    with tc.tile_pool(name="xvpool", bufs=1) as xvpool:
                        wv_sb = xvpool.tile([128, KT, C], BF16)
                        for kt in range(KT):
                            nc.sync.dma_start(out=wv_sb[:, kt, :], in_=wv_d.ap()[kt * 128:(kt + 1) * 128, :])
                        if has_v_bias:
                            vb_sb2 = xvpool.tile([1, C], BF16)
                            nc.sync.dma_start(out=vb_sb2, in_=vb_d.ap())
                        else:
                            vb_sb2 = None
                        qk_mtile(0)
                        qk_mtile(6)
                        for b in range(B_LOC):
                            v_block(b, wv_sb, vb_sb2)
                            mul_eng = nc.vector if pair_i % 8 == 7 else nc.gpsimd
                            pair_i += 1
                            do_pair(0, b, mul_eng)

                    for hp in range(1, H // 2):
                        qk_mtile(hp)
                        qk_mtile(6 + hp)
                        for b in range(B_LOC):
                            mul_eng = nc.vector if pair_i % 8 == 7 else nc.gpsimd
                            pair_i += 1
                            do_pair(hp, b, mul_eng)
                    for item in inflight:
                        attn_back(*item)

            # ---- proj matmul: out[tokens, C] ----
            with tc.tile_pool(name="ps_p", bufs=4, space="PSUM") as ps_p:
                for mt in range(NPAD // 128):
                    rows = min(128, NTOK - mt * 128)
                    if rows <= 0:
                        break
                    ps = [ps_p.tile([128, C // 2], F32, name=f"psp{nt}", tag=f"psp{nt}") for nt in range(2)]
                    for kt in range(KT):
                        for nt in range(2):
                            nc.tensor.matmul(
                                ps[nt],
                                lhsT=aoT_sb[:, kt, mt * 128:(mt + 1) * 128],
                                rhs=wp_sb[:, kt, nt * (C // 2):(nt + 1) * (C // 2)],
                                start=(kt == 0),
                                stop=(kt == KT - 1 and not has_p_bias),
                            )
                    if has_p_bias:
                        for nt in range(2):
                            nc.tensor.matmul(
                                ps[nt],
                                lhsT=ones_row[:, mt * 128:(mt + 1) * 128],
                                rhs=pb_sb[:, nt * (C // 2):(nt + 1) * (C // 2)],
                                start=False, stop=True,
                            )
                    stage = outstage.tile([128, C], F32, tag="stage")
                    for nt in range(2):
                        nc.any.tensor_copy(out=stage[:, nt * (C // 2):(nt + 1) * (C // 2)], in_=ps[nt])
                    nc.sync.dma_start(
                        out=out_d.ap()[mt * 128:mt * 128 + rows, :], in_=stage[0:rows, :]
                    )

    nc.compile()
    return nc


def _rel_pos_index():
    ch, cw = np.meshgrid(np.arange(14), np.arange(14), indexing="ij")
    coords = np.stack([ch, cw]).reshape(2, -1)
    rel = (coords[:, :, None] - coords[:, None, :]).transpose(1, 2, 0).astype(np.int64)
    rel[:, :, 0] += 13
    rel[:, :, 1] += 13
    rel[:, :, 0] *= 27
    idx = np.zeros((N, N), dtype=np.int64)
    idx[1:, 1:] = rel.sum(-1)
    num = 27 * 27 + 3
    idx[0, 0:] = num - 3
    idx[0:, 0] = num - 2
    idx[0, 0] = num - 1
    return idx


def _bf16(a):
    return np.ascontiguousarray(a).astype(ml_dtypes.bfloat16)


def kernel(x, qkv_w, q_bias, v_bias, rel_pos_table, proj_w, proj_b, rel_pos_index):
    x = np.asarray(x, dtype=np.float32)
    qkv_w = np.asarray(qkv_w, dtype=np.float32)
    q_bias = np.asarray(q_bias, dtype=np.float32)
    v_bias = np.asarray(v_bias, dtype=np.float32)
    rel_pos_table = np.asarray(rel_pos_table, dtype=np.float32)
    proj_w = np.asarray(proj_w, dtype=np.float32)
    proj_b = np.asarray(proj_b, dtype=np.float32)
    rel_pos_index = np.asarray(rel_pos_index)

    has_q = bool(np.any(q_bias != 0))
    has_v = bool(np.any(v_bias != 0))
    has_p = bool(np.any(proj_b != 0))

    key = (has_v, has_p, has_q)
    if key not in _CACHE:
        _CACHE[key] = _build(*key)
    nc = _CACHE[key]

    # exp of the transposed per-head bias, laid out as the kernel's score tiles:
    # [h][j-partition 0..127, cols 0:197]=j-chunk0, [0:69, 197:394]=j-chunk1,
    # rows 69:128 of the second chunk are zero (kills padded key rows).
    bias = rel_pos_table[rel_pos_index.reshape(-1)].reshape(N, N, H)  # [i, j, h]
    expb = np.zeros((H // 2, 128, 4 * N), dtype=np.float32)
    eb = np.exp(bias)
    for h in range(H):
        ebT = eb[:, :, h].T  # [j, i]
        base = (h % 2) * 2 * N
        expb[h // 2, :, base:base + N] = ebT[0:128, :]
        expb[h // 2, 0:N - 128, base + N:base + 2 * N] = ebT[128:N, :]
    expb16 = _bf16(expb)

    wqk16 = _bf16(qkv_w[0:2 * C].T)           # [768, 1536]
    wv16 = _bf16(qkv_w[2 * C:3 * C].T)        # [768, 768]
    wp16 = _bf16(proj_w.T)                    # [768, 768]

    in_maps = []
    for c in range(N_CORES):
        xs = x[c * B_LOC:(c + 1) * B_LOC]                      # [8, 197, 768]
        xT = np.zeros((C, NPAD), dtype=ml_dtypes.bfloat16)
        xT[:, 0:NTOK] = _bf16(xs.reshape(NTOK, C).T)
        m = {
            "xT": xT,
            "wqk": wqk16,
            "wv": wv16,
            "wp": wp16,
            "expb": expb16,
        }
        if has_q:
            m["qb"] = np.ascontiguousarray(q_bias.reshape(KT, 128).T)
        if has_v:
            m["vb"] = _bf16(v_bias.reshape(1, C))
        if has_p:
            m["pb"] = _bf16(proj_b.reshape(1, C))
        in_maps.append(m)

    res = bass_utils.run_bass_kernel_spmd(nc, in_maps, core_ids=list(range(N_CORES)))
    out = np.empty((B, N, C), dtype=np.float32)
    for c in range(N_CORES):
        out[c * B_LOC:(c + 1) * B_LOC] = res.results[c]["out"].reshape(B_LOC, N, C)
    return out
